# revision 22
# baseline (speedup 1.0000x reference)
"""Trainium2 Bass kernel for fused multi-head self-attention.

Reference computation (per batch b):
    qkv = (x @ w_qkv.T) -> split into q, k, v heads [H=6, N=2048, D=64]
    scores = q @ k.T * D**-0.5          [H, N, N]
    attn = softmax(scores, axis=-1)     [H, N, N]   (output #2)
    out = (attn @ v).reshape(N, C) @ w_proj.T + b_proj   (output #1)

Sharding: batch (B=8) across the 8 NeuronCores, one batch per core.
No collectives; each core produces out[b] and attn[b], host stacks them.

Per-core dataflow (all matmuls in float32r = full-rate PE):
  phase 0/1: load x, w; PE-transpose to x^T, w_qkv^T, w_proj^T; compute
             qkv^T = w_qkv @ x^T (Q^T, K^T kept transposed [D, N]),
             V kept natural [N, D] interleaved with a ones column.
  phase 2, per head h:
    N-side (per 128-query tile): S = Q_tile @ K^T in PSUM -> ScalarE
      exp(scale*S) with accum_out row-sums -> DVE reciprocal ->
      DVE tensor_scalar in-place normalize -> DMA 1 MiB tile to attn HBM.
    T-side (per 1024-query chunk, accumulate over 16 key tiles):
      S^T = K_tile @ Q^T in PSUM -> ScalarE exp -> PE out' += [V|1]^T @ E^T.
      Row 64 of out' = softmax denominators; broadcast reciprocal via a
      ones-outer-product matmul, normalize to out_heads^T [C, N].
  phase 3: out = out_heads^T.T @ w_proj^T + b_proj (natural layout) -> DMA.
"""

import numpy as np

import concourse.bass as bass
import concourse.mybir as mybir
import concourse.tile as tile
from concourse.masks import make_identity
from concourse.vector_clock import ScopedClock

F32 = mybir.dt.float32
F32R = mybir.dt.float32r
EXP = mybir.ActivationFunctionType.Exp

B, N, C = 8, 2048, 384
H, D = 6, 64
SCALE = float(D) ** -0.5
NT = N // 128          # 16 query/key tiles of 128
NCH = N // 512         # 4 chunks of 512

# this walrus build rejects instructions carrying more than one sync-wait
_MAX_WAITS = 1

# dev-only knobs for simulator bisection; the grading path leaves these False
SKIP_N_DMA = False
SKIP_T_SIDE = False
SKIP_N_SIDE = False
SKIP_PH3 = False

# phase-2 tiling knobs (PSUM bank budget: N_SLOTS*N_CHUNK/512 + T_BUFS*T_CHUNK/512
#  + OUT_BUFS*T_CHUNK/512 banks must stay <= 8)
N_CHUNK = 1024
N_BUFS = 1
T_CHUNK = 1024
T_BUFS = 2
OUT_BUFS = 1


def _patched_drain_and_barrier(self, tick_clock, wait_clock):
    """Tail drain, with its sem waits spread over chained sync NOPs."""
    carrier = self.nc.sync.nop(nofuse=True)
    wait_clock.add_sem_waits(
        carrier.ins, ScopedClock({None: tick_clock.global_clock})
    )
    si = carrier.ins.sync_info
    waits = list(si.on_wait) if si is not None else []
    carrier.ins.sync_info = mybir.SyncInfo(on_wait=waits[:_MAX_WAITS], on_update=[])
    for i in range(_MAX_WAITS, len(waits), _MAX_WAITS):
        nop = self.nc.sync.nop(nofuse=True)
        nop.ins.sync_info = mybir.SyncInfo(
            on_wait=waits[i : i + _MAX_WAITS], on_update=[]
        )
    self.nc.sync.drain()

    self.nc.all_engine_barrier()
    assert self.sems is not None
    popped = self.nc._tile_sem_poison_stack.pop()
    assert popped is self._sem_poison
    self.nc.clear_and_free_semaphores(list(self.sems.allocated().values()))
    self.nc.all_engine_barrier()


tile.TileContext._drain_and_barrier = _patched_drain_and_barrier


def _split_multi_waits(nc):
    """Hoist excess sync-waits onto same-engine NOPs ahead of the owner."""
    n_id = 0
    for f in nc.m.functions:
        for blk in f.blocks:
            insts = blk.instructions
            out = []
            changed = False
            for inst in insts:
                si = inst.sync_info
                if si is not None and len(si.on_wait) > _MAX_WAITS:
                    waits = list(si.on_wait)
                    keep = len(waits) - _MAX_WAITS
                    for i in range(0, keep, _MAX_WAITS):
                        nop = mybir.InstNoOp(
                            name=f"I-waitsplit-{n_id}", ins=[], outs=[]
                        )
                        n_id += 1
                        nop.engine = inst.engine
                        nop.sync_info = mybir.SyncInfo(
                            on_wait=waits[i : i + _MAX_WAITS], on_update=[]
                        )
                        out.append(nop)
                    inst.sync_info = mybir.SyncInfo(
                        on_wait=waits[keep:], on_update=list(si.on_update)
                    )
                    changed = True
                out.append(inst)
            if changed:
                blk.instructions = out


def build_nc():
    nc = bass.Bass("TRN2", target_bir_lowering=False, debug=False, num_devices=8)

    x_d = nc.dram_tensor("x", [N, C], F32, kind="ExternalInput")
    wqkv_d = nc.dram_tensor("w_qkv", [3 * C, C], F32, kind="ExternalInput")
    wproj_d = nc.dram_tensor("w_proj", [C, C], F32, kind="ExternalInput")
    bproj_d = nc.dram_tensor("b_proj", [C], F32, kind="ExternalInput")
    out_d = nc.dram_tensor("out", [N, C], F32, kind="ExternalOutput")
    attn_d = nc.dram_tensor("attn", [H, N, N], F32, kind="ExternalOutput")

    with tile.TileContext(nc) as tc, nc.allow_low_precision(
        reason="float32r tiles feed full-rate PE matmuls; accumulation stays fp32"
    ):
        _build_body(nc, tc, x_d, wqkv_d, wproj_d, bproj_d, out_d, attn_d)
    _split_multi_waits(nc)
    return nc


def _build_body(nc, tc, x_d, wqkv_d, wproj_d, bproj_d, out_d, attn_d):
    import contextlib

    ctx = contextlib.ExitStack()
    with ctx:
        persist = ctx.enter_context(tc.tile_pool(name="persist", bufs=1))

        ident = persist.tile([128, 128], F32)
        make_identity(nc, ident)

        # persistent SBUF tensors
        qkT = [persist.tile([128, N], F32R, name=f"qkT{m}", tag=f"qkT{m}") for m in range(6)]
        v_sb = [persist.tile([128, H * (D + 1)], F32R, name=f"v{t}", tag=f"v{t}") for t in range(NT)]
        oT = [persist.tile([128, N], F32R, name=f"oTt{m}", tag=f"oTt{m}") for m in range(3)]
        wqkvT = [persist.tile([128, 3 * C], F32R, name=f"wqkvT{m}", tag=f"wqkvT{m}") for m in range(3)]
        wprojT = [persist.tile([128, C], F32R, name=f"wprojT{m}", tag=f"wprojT{m}") for m in range(3)]
        bias_bc = persist.tile([128, C], F32)
        ones64_f32 = persist.tile([1, D], F32)
        nc.vector.memset(ones64_f32, 1.0)
        ones64 = persist.tile([1, D], F32R)
        nc.vector.tensor_copy(ones64, ones64_f32)
        ones6 = persist.tile([128, H], F32)
        nc.vector.memset(ones6, 1.0)
        for t in range(NT):
            nc.vector.tensor_copy(
                v_sb[t].rearrange("p (h e) -> p h e", h=H)[:, :, D : D + 1],
                ones6[:, :, None],
            )

        # one unified PSUM budget (8 banks):
        #   s_nat tag: [128, N_CHUNK] x N_BUFS      (+ phase-1 V/bias tiles)
        #   s_t tag:   [128, T_CHUNK] x T_BUFS      (+ phase-1 qk tiles, rb, proj)
        #   outp tag:  [65, T_CHUNK] x OUT_BUFS     (+ phase-1 transpose tiles)
        eN_pool = ctx.enter_context(tc.tile_pool(name="eN", bufs=3))
        eT_pool = ctx.enter_context(tc.tile_pool(name="eT", bufs=3))
        small = ctx.enter_context(tc.tile_pool(name="small", bufs=8))
        rowT_pool = ctx.enter_context(tc.tile_pool(name="rowT", bufs=1))
        o_pool = ctx.enter_context(tc.tile_pool(name="o_sb", bufs=3))
        ph1_sb = ctx.enter_context(tc.tile_pool(name="ph1_sb", bufs=2))
        xT_pool = ctx.enter_context(tc.tile_pool(name="xT_pool", bufs=1))

        xT = [xT_pool.tile([128, N], F32R, name=f"xTt{m}", tag=f"xTt{m}") for m in range(3)]

        NQC = N // T_CHUNK          # T-side q chunks
        NSUB = N // N_CHUNK         # N-side key sub-chunks per query tile
        QT_PER_QC = T_CHUNK // 128  # query tiles per q chunk

        def w_transpose(m, dest, pool, tag):
            w_nat = ph1_sb.tile([128, C], F32, tag="w_nat", name=f"w_nat_{id(dest)}_{m}")
            src_d = wqkv_d if dest is wqkvT else wproj_d
            nc.sync.dma_start(out=w_nat, in_=src_d.ap()[m * 128 : (m + 1) * 128, :])
            for cc in range(3):
                tp = pool.tile([128, 128], F32, tag=tag, name=f"tp_{id(dest)}_{m}_{cc}")
                nc.tensor.transpose(tp, w_nat[:, cc * 128 : (cc + 1) * 128], ident)
                nc.scalar.copy(out=dest[cc][:, m * 128 : (m + 1) * 128], in_=tp)

        def qk_tile(m, pool, tag):
            for nch in range(NCH):
                mm = pool.tile([128, 512], F32, tag=tag, name=f"qk_{m}_{nch}")
                for cc in range(3):
                    nc.tensor.matmul(
                        mm,
                        wqkvT[cc][:, m * 128 : (m + 1) * 128],
                        xT[cc][:, nch * 512 : (nch + 1) * 512],
                        start=(cc == 0),
                        stop=(cc == 2),
                    )
                nc.vector.tensor_copy(qkT[m][:, nch * 512 : (nch + 1) * 512], mm)

        def x_transposes(pool, tag):
            for g in range(NT // 4):
                xn = []
                for i in range(4):
                    t = 4 * g + i
                    x_nat = ph1_sb.tile([128, C], F32, tag="x_nat", name=f"x_nat_{t}", bufs=5)
                    nc.sync.dma_start(
                        out=x_nat, in_=x_d.ap()[t * 128 : (t + 1) * 128, :]
                    )
                    xn.append(x_nat)
                for cc in range(3):
                    tp4 = pool.tile([128, 512], F32, tag=tag, name=f"tp4_{g}_{cc}")
                    for i in range(4):
                        nc.tensor.transpose(
                            tp4[:, i * 128 : (i + 1) * 128],
                            xn[i][:, cc * 128 : (cc + 1) * 128],
                            ident,
                        )
                    nc.scalar.copy(out=xT[cc][:, g * 512 : (g + 1) * 512], in_=tp4)

        def v_tiles(pool, tag):
            for t in range(NT):
                mm = pool.tile([128, C], F32, tag=tag, name=f"vps_{t}")
                for cc in range(3):
                    nc.tensor.matmul(
                        mm,
                        xT[cc][:, t * 128 : (t + 1) * 128],
                        wqkvT[cc][:, 2 * C : 3 * C],
                        start=(cc == 0),
                        stop=(cc == 2),
                    )
                for h in range(H):
                    nc.vector.tensor_copy(
                        v_sb[t][:, h * (D + 1) : h * (D + 1) + D],
                        mm[:, h * D : (h + 1) * D],
                    )

        def bias_broadcast(pool, tag):
            b_row = ph1_sb.tile([1, C], F32, tag="b_row")
            nc.sync.dma_start(out=b_row, in_=bproj_d.ap()[None, :])
            ones128 = ph1_sb.tile([1, 128], F32, tag="ones128")
            nc.vector.memset(ones128, 1.0)
            bb_ps = pool.tile([128, C], F32, tag=tag, name="bb_ps")
            nc.tensor.matmul(bb_ps, ones128, b_row, start=True, stop=True)
            nc.vector.tensor_copy(bias_bc, bb_ps)

        def n_side(h, qi):
            qm, qo = h // 2, 64 * (h % 2)
            q_lhs, k_lhs = qkT[qm], qkT[3 + h // 2]
            e_nat = eN_pool.tile([128, N], F32, tag="e_nat")
            subsums = []
            for sub in range(NSUB):
                s_nat = pN.tile([128, N_CHUNK], F32, tag="s_nat")
                for kc in range(N_CHUNK // 512):
                    k0 = sub * N_CHUNK + kc * 512
                    nc.tensor.matmul(
                        s_nat[:, kc * 512 : (kc + 1) * 512],
                        q_lhs[qo : qo + D, qi * 128 : (qi + 1) * 128],
                        k_lhs[qo : qo + D, k0 : k0 + 512],
                        start=True,
                        stop=True,
                    )
                ss = small.tile([128, 1], F32, tag=f"ss{sub}")
                subsums.append(ss)
                nc.scalar.activation(
                    out=e_nat[:, sub * N_CHUNK : (sub + 1) * N_CHUNK],
                    in_=s_nat,
                    func=EXP,
                    scale=SCALE,
                    accum_out=ss,
                )
            sums = small.tile([128, 1], F32, tag="sums")
            if NSUB == 1:
                sums = subsums[0]
            elif NSUB == 2:
                nc.vector.tensor_tensor(
                    out=sums, in0=subsums[0], in1=subsums[1],
                    op=mybir.AluOpType.add,
                )
            else:
                raise NotImplementedError
            recip = small.tile([128, 1], F32, tag="recip")
            nc.vector.reciprocal(out=recip, in_=sums)
            nc.vector.tensor_scalar_mul(e_nat, e_nat, recip)
            if not SKIP_N_DMA:
                nc.sync.dma_start(
                    out=attn_d.ap()[h, qi * 128 : (qi + 1) * 128, :], in_=e_nat
                )

        def t_side(h, qc, ki, outp):
            qm, qo = h // 2, 64 * (h % 2)
            q_lhs, k_lhs = qkT[qm], qkT[3 + h // 2]
            s_t = pT.tile([128, T_CHUNK], F32, tag="s_t")
            for j in range(T_CHUNK // 512):
                q0 = qc * T_CHUNK + j * 512
                nc.tensor.matmul(
                    s_t[:, j * 512 : (j + 1) * 512],
                    k_lhs[qo : qo + D, ki * 128 : (ki + 1) * 128],
                    q_lhs[qo : qo + D, q0 : q0 + 512],
                    start=True,
                    stop=True,
                )
            e_t = eT_pool.tile([128, T_CHUNK], F32R, tag="e_t")
            nc.scalar.activation(out=e_t, in_=s_t, func=EXP, scale=SCALE)
            for j in range(T_CHUNK // 512):
                nc.tensor.matmul(
                    outp[:, j * 512 : (j + 1) * 512],
                    v_sb[ki][:, h * (D + 1) : (h + 1) * (D + 1)],
                    e_t[:, j * 512 : (j + 1) * 512],
                    start=(ki == 0),
                    stop=(ki == NT - 1),
                )

        def t_finish(h, qc, outp):
            recip_t = rowT_pool.tile([1, T_CHUNK], F32R, tag="recip_t")
            nc.vector.reciprocal(out=recip_t, in_=outp[D : D + 1, :])
            om, oo = h // 2, 64 * (h % 2)
            for j in range(T_CHUNK // 512):
                rb = pT.tile([64, 512], F32, tag="s_t")
                nc.tensor.matmul(
                    rb,
                    ones64,
                    recip_t[:, j * 512 : (j + 1) * 512],
                    start=True,
                    stop=True,
                )
                rb_sb = rowT_pool.tile([64, 512], F32, tag="rb_sb")
                nc.vector.tensor_copy(rb_sb, rb)
                q0 = qc * T_CHUNK + j * 512
                nc.vector.tensor_tensor(
                    out=oT[om][oo : oo + D, q0 : q0 + 512],
                    in0=outp[0:D, j * 512 : (j + 1) * 512],
                    in1=rb_sb,
                    op=mybir.AluOpType.mult,
                )

        def proj(qi):
            mm = pT.tile([128, C], F32, tag="s_t", name=f"proj_{qi}")
            for cc in range(3):
                nc.tensor.matmul(
                    mm,
                    oT[cc][:, qi * 128 : (qi + 1) * 128],
                    wprojT[cc],
                    start=(cc == 0),
                    stop=(cc == 2),
                )
            o_sb = o_pool.tile([128, C], F32, tag="o_sb")
            nc.vector.tensor_tensor(
                out=o_sb, in0=mm, in1=bias_bc, op=mybir.AluOpType.add
            )
            nc.sync.dma_start(
                out=out_d.ap()[qi * 128 : (qi + 1) * 128, :], in_=o_sb
            )

        def attention(h_pair, emit_proj, fillers=None):
            for qc in range(NQC):
                if fillers:
                    for fn in fillers.get(qc, []):
                        fn()
                for h in h_pair:
                    outp = None if SKIP_T_SIDE else pO.tile(
                        [D + 1, T_CHUNK], F32, tag="outp",
                        name=f"outp_{qc}_{h}",
                    )
                    ki_per_step = (NT + QT_PER_QC - 1) // QT_PER_QC
                    for step in range(QT_PER_QC):
                        if not SKIP_T_SIDE:
                            for u in range(ki_per_step):
                                ki = step * ki_per_step + u
                                if ki < NT:
                                    t_side(h, qc, ki, outp)
                        if not SKIP_N_SIDE:
                            n_side(h, qc * QT_PER_QC + step)
                    if not SKIP_T_SIDE:
                        t_finish(h, qc, outp)
                if emit_proj and not (SKIP_PH3 or SKIP_T_SIDE):
                    for step in range(QT_PER_QC):
                        proj(qc * QT_PER_QC + step)

        # ---- bootstrap: x^T, w^T, QKV projection (own PSUM scope) ----
        with tc.tile_pool(name="boot_ps", bufs=2, space="PSUM") as boot:
            for m in [0, 3, 6, 7, 8]:
                w_transpose(m, wqkvT, boot, "btp")
            x_transposes(boot, "btp4")
            qk_tile(0, boot, "bqk")
            qk_tile(3, boot, "bqk")
            v_tiles(boot, "bv")
            bias_broadcast(boot, "bv")
            for m in [1, 4, 2, 5]:
                w_transpose(m, wqkvT, boot, "btp")
                qk_tile(m, boot, "bqk")
            for m in range(3):
                w_transpose(m, wprojT, boot, "btp")

        pN = ctx.enter_context(tc.tile_pool(name="sN", bufs=N_BUFS, space="PSUM"))
        pT = ctx.enter_context(tc.tile_pool(name="sT", bufs=T_BUFS, space="PSUM"))
        pO = ctx.enter_context(tc.tile_pool(name="sO", bufs=OUT_BUFS, space="PSUM"))

        attention(list(range(H)), emit_proj=True)


_NC_CACHE = None


def kernel(x, w_qkv, w_proj, b_proj):
    global _NC_CACHE
    from concourse.bass_utils import run_bass_kernel_spmd

    x = np.ascontiguousarray(np.asarray(x, dtype=np.float32))
    w_qkv = np.ascontiguousarray(np.asarray(w_qkv, dtype=np.float32))
    w_proj = np.ascontiguousarray(np.asarray(w_proj, dtype=np.float32))
    b_proj = np.ascontiguousarray(np.asarray(b_proj, dtype=np.float32))

    if _NC_CACHE is None:
        _NC_CACHE = build_nc()
    nc = _NC_CACHE

    in_maps = [
        {
            "x": np.ascontiguousarray(x[b]),
            "w_qkv": w_qkv,
            "w_proj": w_proj,
            "b_proj": b_proj,
        }
        for b in range(B)
    ]
    res = run_bass_kernel_spmd(nc, in_maps, core_ids=list(range(B)))
    out = np.stack([r["out"] for r in res.results])
    attn = np.stack([r["attn"] for r in res.results])
    return out, attn


# revision 27
# speedup vs baseline: 1.0159x; 1.0159x over previous
"""Trainium2 Bass kernel for fused multi-head self-attention (B=8, H=6, N=2048, C=384).

Outputs (matching the reference): out [B, N, C] and the full normalized
attention matrix attn [B, H, N, N] (fp32, ~805 MB -- the memory-bound part).

Sharding: batch across the 8 NeuronCores (data parallel, no collectives);
each core computes out[b] and attn[b]; the host stacks results.

Per-core dataflow (matmuls in float32r = full-rate PE, fp32 accumulate):
  boot:  load x, w; PE-transpose into x^T, w_qkv^T, w_proj^T; Q^T/K^T =
         w_qkv[:2C] @ x^T kept transposed [D, N]; V natural [N, D] stored
         per key-tile interleaved with a ones column ([V_h | 1]).
  attention, per (q-chunk, head):
    N-side (per 128-query tile): S = Q_tile @ K^T in PSUM -> ScalarE
      exp(scale*S) with accum_out row-sums -> DVE reciprocal -> DVE
      tensor_scalar normalize in place -> 1 MiB contiguous DMA to attn HBM.
    T-side (accumulating over 16 key tiles): S^T = K_tile @ Q^T -> ScalarE
      exp -> PE out' += [V_h|1]^T @ E^T.  Row D of out' is the softmax
      denominator row; a ones-outer-product matmul broadcasts its
      reciprocal and DVE scales rows 0..D-1 into out_heads^T [C, N].
    projection (per q chunk, overlapped): out = out_heads^T.T @ w_proj^T
      + b_proj in natural layout -> DMA.

Engine budget per core (cost model): ScalarE exp streams ~460 us (the
bottleneck: the attention matrix is exponentiated twice, once per
orientation, because PE matmuls contract over partitions and HBM needs
row-major attn), DMA ~304 us, PE ~293 us, DVE ~199 us; modeled e2e 517 us.

Environment workarounds baked in:
  * this walrus build accepts at most ONE sync-wait per instruction:
    _split_multi_waits() hoists extras onto same-engine NOPs, and the
    patched TileContext._drain_and_barrier does the same for the tail drain;
  * float32r operands must be written by "rounding producers" (copy/
    activation outputs with float32r dtype), not plain bitcasts.
"""

import numpy as np

import concourse.bass as bass
import concourse.mybir as mybir
import concourse.tile as tile
from concourse.masks import make_identity
from concourse.vector_clock import ScopedClock

F32 = mybir.dt.float32
F32R = mybir.dt.float32r
EXP = mybir.ActivationFunctionType.Exp

B, N, C = 8, 2048, 384
H, D = 6, 64
SCALE = float(D) ** -0.5
NT = N // 128          # 16 query/key tiles of 128
NCH = N // 512         # 4 chunks of 512

# this walrus build rejects instructions carrying more than one sync-wait
_MAX_WAITS = 1

# dev-only knobs for simulator bisection; the grading path leaves these False
SKIP_N_DMA = False
SKIP_T_SIDE = False
SKIP_N_SIDE = False
SKIP_PH3 = False

# phase-2 tiling knobs (PSUM bank budget: N_SLOTS*N_CHUNK/512 + T_BUFS*T_CHUNK/512
#  + OUT_BUFS*T_CHUNK/512 banks must stay <= 8)
N_CHUNK = 1024
N_BUFS = 1
T_CHUNK = 1024
T_BUFS = 2
OUT_BUFS = 1


def _patched_drain_and_barrier(self, tick_clock, wait_clock):
    """Tail drain, with its sem waits spread over chained sync NOPs."""
    carrier = self.nc.sync.nop(nofuse=True)
    wait_clock.add_sem_waits(
        carrier.ins, ScopedClock({None: tick_clock.global_clock})
    )
    si = carrier.ins.sync_info
    waits = list(si.on_wait) if si is not None else []
    carrier.ins.sync_info = mybir.SyncInfo(on_wait=waits[:_MAX_WAITS], on_update=[])
    for i in range(_MAX_WAITS, len(waits), _MAX_WAITS):
        nop = self.nc.sync.nop(nofuse=True)
        nop.ins.sync_info = mybir.SyncInfo(
            on_wait=waits[i : i + _MAX_WAITS], on_update=[]
        )
    self.nc.sync.drain()

    self.nc.all_engine_barrier()
    assert self.sems is not None
    popped = self.nc._tile_sem_poison_stack.pop()
    assert popped is self._sem_poison
    self.nc.clear_and_free_semaphores(list(self.sems.allocated().values()))
    self.nc.all_engine_barrier()


tile.TileContext._drain_and_barrier = _patched_drain_and_barrier


def _split_multi_waits(nc):
    """Hoist excess sync-waits onto same-engine NOPs ahead of the owner."""
    n_id = 0
    for f in nc.m.functions:
        for blk in f.blocks:
            insts = blk.instructions
            out = []
            changed = False
            for inst in insts:
                si = inst.sync_info
                if si is not None and len(si.on_wait) > _MAX_WAITS:
                    waits = list(si.on_wait)
                    keep = len(waits) - _MAX_WAITS
                    for i in range(0, keep, _MAX_WAITS):
                        nop = mybir.InstNoOp(
                            name=f"I-waitsplit-{n_id}", ins=[], outs=[]
                        )
                        n_id += 1
                        nop.engine = inst.engine
                        nop.sync_info = mybir.SyncInfo(
                            on_wait=waits[i : i + _MAX_WAITS], on_update=[]
                        )
                        out.append(nop)
                    inst.sync_info = mybir.SyncInfo(
                        on_wait=waits[keep:], on_update=list(si.on_update)
                    )
                    changed = True
                out.append(inst)
            if changed:
                blk.instructions = out


def build_nc():
    nc = bass.Bass("TRN2", target_bir_lowering=False, debug=False, num_devices=8)

    x_d = nc.dram_tensor("x", [N, C], F32, kind="ExternalInput")
    wqkv_d = nc.dram_tensor("w_qkv", [3 * C, C], F32, kind="ExternalInput")
    wproj_d = nc.dram_tensor("w_proj", [C, C], F32, kind="ExternalInput")
    bproj_d = nc.dram_tensor("b_proj", [C], F32, kind="ExternalInput")
    out_d = nc.dram_tensor("out", [N, C], F32, kind="ExternalOutput")
    attn_d = nc.dram_tensor("attn", [H, N, N], F32, kind="ExternalOutput")

    with tile.TileContext(nc) as tc, nc.allow_low_precision(
        reason="float32r tiles feed full-rate PE matmuls; accumulation stays fp32"
    ):
        _build_body(nc, tc, x_d, wqkv_d, wproj_d, bproj_d, out_d, attn_d)
    _split_multi_waits(nc)
    return nc


def _build_body(nc, tc, x_d, wqkv_d, wproj_d, bproj_d, out_d, attn_d):
    import contextlib

    ctx = contextlib.ExitStack()
    with ctx:
        persist = ctx.enter_context(tc.tile_pool(name="persist", bufs=1))

        ident = persist.tile([128, 128], F32)
        make_identity(nc, ident)

        # persistent SBUF tensors
        qkT = [persist.tile([128, N], F32R, name=f"qkT{m}", tag=f"qkT{m}") for m in range(6)]
        v_sb = [persist.tile([128, H * (D + 1)], F32R, name=f"v{t}", tag=f"v{t}") for t in range(NT)]
        oT = [persist.tile([128, N], F32R, name=f"oTt{m}", tag=f"oTt{m}") for m in range(3)]
        wqkvT = [persist.tile([128, 3 * C], F32R, name=f"wqkvT{m}", tag=f"wqkvT{m}") for m in range(3)]
        wprojT = [persist.tile([128, C], F32R, name=f"wprojT{m}", tag=f"wprojT{m}") for m in range(3)]
        bias_bc = persist.tile([128, C], F32)
        ones64_f32 = persist.tile([1, D], F32)
        nc.vector.memset(ones64_f32, 1.0)
        ones64 = persist.tile([1, D], F32R)
        nc.vector.tensor_copy(ones64, ones64_f32)
        ones6 = persist.tile([128, H], F32)
        nc.vector.memset(ones6, 1.0)
        for t in range(NT):
            nc.vector.tensor_copy(
                v_sb[t].rearrange("p (h e) -> p h e", h=H)[:, :, D : D + 1],
                ones6[:, :, None],
            )

        # one unified PSUM budget (8 banks):
        #   s_nat tag: [128, N_CHUNK] x N_BUFS      (+ phase-1 V/bias tiles)
        #   s_t tag:   [128, T_CHUNK] x T_BUFS      (+ phase-1 qk tiles, rb, proj)
        #   outp tag:  [65, T_CHUNK] x OUT_BUFS     (+ phase-1 transpose tiles)
        eN_pool = ctx.enter_context(tc.tile_pool(name="eN", bufs=3))
        eT_pool = ctx.enter_context(tc.tile_pool(name="eT", bufs=3))
        small = ctx.enter_context(tc.tile_pool(name="small", bufs=8))
        rowT_pool = ctx.enter_context(tc.tile_pool(name="rowT", bufs=1))
        o_pool = ctx.enter_context(tc.tile_pool(name="o_sb", bufs=3))
        ph1_sb = ctx.enter_context(tc.tile_pool(name="ph1_sb", bufs=2))
        xT_pool = ctx.enter_context(tc.tile_pool(name="xT_pool", bufs=1))

        xT = [xT_pool.tile([128, N], F32R, name=f"xTt{m}", tag=f"xTt{m}") for m in range(3)]

        NQC = N // T_CHUNK          # T-side q chunks
        NSUB = N // N_CHUNK         # N-side key sub-chunks per query tile
        QT_PER_QC = T_CHUNK // 128  # query tiles per q chunk

        def w_transpose(m, dest, pool, tag):
            w_nat = ph1_sb.tile([128, C], F32, tag="w_nat", name=f"w_nat_{id(dest)}_{m}")
            src_d = wqkv_d if dest is wqkvT else wproj_d
            nc.sync.dma_start(out=w_nat, in_=src_d.ap()[m * 128 : (m + 1) * 128, :])
            for cc in range(3):
                tp = pool.tile([128, 128], F32, tag=tag, name=f"tp_{id(dest)}_{m}_{cc}")
                nc.tensor.transpose(tp, w_nat[:, cc * 128 : (cc + 1) * 128], ident)
                nc.scalar.copy(out=dest[cc][:, m * 128 : (m + 1) * 128], in_=tp)

        def qk_tile(m, pool, tag):
            for nch in range(NCH):
                mm = pool.tile([128, 512], F32, tag=tag, name=f"qk_{m}_{nch}")
                for cc in range(3):
                    nc.tensor.matmul(
                        mm,
                        wqkvT[cc][:, m * 128 : (m + 1) * 128],
                        xT[cc][:, nch * 512 : (nch + 1) * 512],
                        start=(cc == 0),
                        stop=(cc == 2),
                    )
                nc.scalar.copy(out=qkT[m][:, nch * 512 : (nch + 1) * 512], in_=mm)

        def x_transposes(pool, tag):
            for g in range(NT // 4):
                xn = []
                for i in range(4):
                    t = 4 * g + i
                    x_nat = ph1_sb.tile([128, C], F32, tag="x_nat", name=f"x_nat_{t}", bufs=5)
                    nc.sync.dma_start(
                        out=x_nat, in_=x_d.ap()[t * 128 : (t + 1) * 128, :]
                    )
                    xn.append(x_nat)
                for cc in range(3):
                    tp4 = pool.tile([128, 512], F32, tag=tag, name=f"tp4_{g}_{cc}")
                    for i in range(4):
                        nc.tensor.transpose(
                            tp4[:, i * 128 : (i + 1) * 128],
                            xn[i][:, cc * 128 : (cc + 1) * 128],
                            ident,
                        )
                    nc.scalar.copy(out=xT[cc][:, g * 512 : (g + 1) * 512], in_=tp4)

        def v_tiles(pool, tag):
            for t in range(NT):
                mm = pool.tile([128, C], F32, tag=tag, name=f"vps_{t}")
                for cc in range(3):
                    nc.tensor.matmul(
                        mm,
                        xT[cc][:, t * 128 : (t + 1) * 128],
                        wqkvT[cc][:, 2 * C : 3 * C],
                        start=(cc == 0),
                        stop=(cc == 2),
                    )
                for h in range(H):
                    nc.vector.tensor_copy(
                        v_sb[t][:, h * (D + 1) : h * (D + 1) + D],
                        mm[:, h * D : (h + 1) * D],
                    )

        def bias_broadcast(pool, tag):
            b_row = ph1_sb.tile([1, C], F32, tag="b_row")
            nc.sync.dma_start(out=b_row, in_=bproj_d.ap()[None, :])
            ones128 = ph1_sb.tile([1, 128], F32, tag="ones128")
            nc.vector.memset(ones128, 1.0)
            bb_ps = pool.tile([128, C], F32, tag=tag, name="bb_ps")
            nc.tensor.matmul(bb_ps, ones128, b_row, start=True, stop=True)
            nc.vector.tensor_copy(bias_bc, bb_ps)

        def n_side(h, qi):
            qm, qo = h // 2, 64 * (h % 2)
            q_lhs, k_lhs = qkT[qm], qkT[3 + h // 2]
            e_nat = eN_pool.tile([128, N], F32, tag="e_nat")
            subsums = []
            for sub in range(NSUB):
                s_nat = pN.tile([128, N_CHUNK], F32, tag="s_nat")
                for kc in range(N_CHUNK // 512):
                    k0 = sub * N_CHUNK + kc * 512
                    nc.tensor.matmul(
                        s_nat[:, kc * 512 : (kc + 1) * 512],
                        q_lhs[qo : qo + D, qi * 128 : (qi + 1) * 128],
                        k_lhs[qo : qo + D, k0 : k0 + 512],
                        start=True,
                        stop=True,
                    )
                ss = small.tile([128, 1], F32, tag=f"ss{sub}")
                subsums.append(ss)
                nc.scalar.activation(
                    out=e_nat[:, sub * N_CHUNK : (sub + 1) * N_CHUNK],
                    in_=s_nat,
                    func=EXP,
                    scale=SCALE,
                    accum_out=ss,
                )
            sums = small.tile([128, 1], F32, tag="sums")
            if NSUB == 1:
                sums = subsums[0]
            elif NSUB == 2:
                nc.vector.tensor_tensor(
                    out=sums, in0=subsums[0], in1=subsums[1],
                    op=mybir.AluOpType.add,
                )
            else:
                raise NotImplementedError
            recip = small.tile([128, 1], F32, tag="recip")
            nc.vector.reciprocal(out=recip, in_=sums)
            nc.vector.tensor_scalar_mul(e_nat, e_nat, recip)
            if not SKIP_N_DMA:
                nc.sync.dma_start(
                    out=attn_d.ap()[h, qi * 128 : (qi + 1) * 128, :], in_=e_nat
                )

        def t_side(h, qc, ki, outp):
            qm, qo = h // 2, 64 * (h % 2)
            q_lhs, k_lhs = qkT[qm], qkT[3 + h // 2]
            s_t = pT.tile([128, T_CHUNK], F32, tag="s_t")
            for j in range(T_CHUNK // 512):
                q0 = qc * T_CHUNK + j * 512
                nc.tensor.matmul(
                    s_t[:, j * 512 : (j + 1) * 512],
                    k_lhs[qo : qo + D, ki * 128 : (ki + 1) * 128],
                    q_lhs[qo : qo + D, q0 : q0 + 512],
                    start=True,
                    stop=True,
                )
            e_t = eT_pool.tile([128, T_CHUNK], F32R, tag="e_t")
            nc.scalar.activation(out=e_t, in_=s_t, func=EXP, scale=SCALE)
            for j in range(T_CHUNK // 512):
                nc.tensor.matmul(
                    outp[:, j * 512 : (j + 1) * 512],
                    v_sb[ki][:, h * (D + 1) : (h + 1) * (D + 1)],
                    e_t[:, j * 512 : (j + 1) * 512],
                    start=(ki == 0),
                    stop=(ki == NT - 1),
                )

        def t_finish(h, qc, outp):
            recip_t = rowT_pool.tile([1, T_CHUNK], F32R, tag="recip_t")
            nc.vector.reciprocal(out=recip_t, in_=outp[D : D + 1, :])
            om, oo = h // 2, 64 * (h % 2)
            for j in range(T_CHUNK // 512):
                rb = pT.tile([64, 512], F32, tag="s_t")
                nc.tensor.matmul(
                    rb,
                    ones64,
                    recip_t[:, j * 512 : (j + 1) * 512],
                    start=True,
                    stop=True,
                )
                rb_sb = rowT_pool.tile([64, 512], F32, tag="rb_sb")
                nc.vector.tensor_copy(rb_sb, rb)
                q0 = qc * T_CHUNK + j * 512
                nc.vector.tensor_tensor(
                    out=oT[om][oo : oo + D, q0 : q0 + 512],
                    in0=outp[0:D, j * 512 : (j + 1) * 512],
                    in1=rb_sb,
                    op=mybir.AluOpType.mult,
                )

        def proj(qi):
            mm = pT.tile([128, C], F32, tag="s_t", name=f"proj_{qi}")
            for cc in range(3):
                nc.tensor.matmul(
                    mm,
                    oT[cc][:, qi * 128 : (qi + 1) * 128],
                    wprojT[cc],
                    start=(cc == 0),
                    stop=(cc == 2),
                )
            o_sb = o_pool.tile([128, C], F32, tag="o_sb")
            nc.vector.tensor_tensor(
                out=o_sb, in0=mm, in1=bias_bc, op=mybir.AluOpType.add
            )
            nc.sync.dma_start(
                out=out_d.ap()[qi * 128 : (qi + 1) * 128, :], in_=o_sb
            )

        def attention(h_pair, emit_proj, fillers=None):
            for qc in range(NQC):
                if fillers:
                    for fn in fillers.get(qc, []):
                        fn()
                for h in h_pair:
                    outp = None if SKIP_T_SIDE else pO.tile(
                        [D + 1, T_CHUNK], F32, tag="outp",
                        name=f"outp_{qc}_{h}",
                    )
                    ki_per_step = (NT + QT_PER_QC - 1) // QT_PER_QC
                    for step in range(QT_PER_QC):
                        if not SKIP_N_SIDE:
                            n_side(h, qc * QT_PER_QC + step)
                        if not SKIP_T_SIDE:
                            for u in range(ki_per_step):
                                ki = step * ki_per_step + u
                                if ki < NT:
                                    t_side(h, qc, ki, outp)
                    if not SKIP_T_SIDE:
                        t_finish(h, qc, outp)
                if emit_proj and not (SKIP_PH3 or SKIP_T_SIDE):
                    for step in range(QT_PER_QC):
                        proj(qc * QT_PER_QC + step)

        # ---- bootstrap: x^T, w^T, QKV projection (own PSUM scope) ----
        with tc.tile_pool(name="boot_ps", bufs=2, space="PSUM") as boot:
            for m in [0, 3, 6, 7, 8]:
                w_transpose(m, wqkvT, boot, "btp")
            x_transposes(boot, "btp4")
            qk_tile(0, boot, "bqk")
            qk_tile(3, boot, "bqk")
            v_tiles(boot, "bv")
            bias_broadcast(boot, "bv")
            for m in [1, 4, 2, 5]:
                w_transpose(m, wqkvT, boot, "btp")
                qk_tile(m, boot, "bqk")
            for m in range(3):
                w_transpose(m, wprojT, boot, "btp")

        pN = ctx.enter_context(tc.tile_pool(name="sN", bufs=N_BUFS, space="PSUM"))
        pT = ctx.enter_context(tc.tile_pool(name="sT", bufs=T_BUFS, space="PSUM"))
        pO = ctx.enter_context(tc.tile_pool(name="sO", bufs=OUT_BUFS, space="PSUM"))

        attention(list(range(H)), emit_proj=True)


_NC_CACHE = None


def kernel(x, w_qkv, w_proj, b_proj):
    global _NC_CACHE
    from concourse.bass_utils import run_bass_kernel_spmd

    x = np.ascontiguousarray(np.asarray(x, dtype=np.float32))
    w_qkv = np.ascontiguousarray(np.asarray(w_qkv, dtype=np.float32))
    w_proj = np.ascontiguousarray(np.asarray(w_proj, dtype=np.float32))
    b_proj = np.ascontiguousarray(np.asarray(b_proj, dtype=np.float32))

    if _NC_CACHE is None:
        _NC_CACHE = build_nc()
    nc = _NC_CACHE

    in_maps = [
        {
            "x": np.ascontiguousarray(x[b]),
            "w_qkv": w_qkv,
            "w_proj": w_proj,
            "b_proj": b_proj,
        }
        for b in range(B)
    ]
    res = run_bass_kernel_spmd(nc, in_maps, core_ids=list(range(B)))
    out = np.stack([r["out"] for r in res.results])
    attn = np.stack([r["attn"] for r in res.results])
    return out, attn
